# revision 47
# baseline (speedup 1.0000x reference)
"""DenseFp8 forward kernel for 8 Trainium2 NeuronCores.

Reference semantics (per reference.py):
  qin  = qdq(x, s_in, 448, e4m3)        # round(x/s_in) on the e4m3 grid, * s_in
  qker = qdq(kernel, s_k, 448, e4m3)
  out  = qin @ qker + bias
  new_hist/new_scale bookkeeping from amax(|x|), amax(|kernel|)

Strategy:
  - 4 row-groups x 2 col-groups over the 8 cores. Core i handles x rows
    [r*2048,(r+1)*2048) and kernel cols [c*2048,(c+1)*2048), r=i//2, c=i%2.
  - Host pre-transposes each x row shard to xT [4096, 2048] so the
    contraction dim lands on SBUF partitions (TensorE matmul contracts
    over partitions for both operands).
  - On device: W f32 chunks stream in, are scaled by 1/s_k and cast to
    fp8e4 (TRN e4m3 == OCP e4m3fn on |v|<=240) into an SBUF-resident
    [128, 32, 2048] buffer.  x streams per 128-row m-tile, same
    conversion.  fp8 DoubleRow matmuls accumulate into PSUM.  Eviction:
    ScalarE copy * (s_in*s_k), VectorE adds the (host-broadcast) bias.
  - amax partials per partition are reduced on VectorE (abs_max) and
    shipped out as [128, 2]; final max + history roll happen on host.
  - Loads are emitted in a diagonal-wave order (W n-slice leads, then two
    x m-blocks per wave) with matmul groups emitted as soon as both of
    their operands are in flight, so the PE ramps up during the load
    phase instead of waiting for full residency.

Measured on 8 axon-tunneled trn2 cores: ~312-327 us HW exec (core 0),
out rel err ~6.8e-05 vs the fp32 jax reference, scale/history outputs
bit-exact.  PE matmul busy time is ~225-231 us of that (~98% of the
157 TF/s fp8 DoubleRow roofline while streaming).
"""

import sys

if "/opt/trn_rl_repo" not in sys.path:
    sys.path.insert(0, "/opt/trn_rl_repo")

import numpy as np

P = 128
N_ROWS = 8192   # x rows
D = 4096        # contraction
U = 4096        # output cols
RG = 4          # row groups
CG = 2          # col groups
M = N_ROWS // RG      # 2048 rows per core
NC_ = U // CG         # 2048 cols per core
KO = D // P           # 32 k-chunks
MT = M // P           # 16 m-tiles
NB = NC_ // 512       # 4 psum banks per m-tile

AMAX_HIS_LEN = 16
E4M3_MAX = 448.0

_cached = {}


def _build():
    import concourse.mybir as mybir
    import concourse.tile as tile
    from concourse import bacc

    f32 = mybir.dt.float32
    fp8 = mybir.dt.float8e4

    nc = bacc.Bacc("TRN2", target_bir_lowering=False, debug=False, num_devices=8)

    xt = nc.dram_tensor("xt", [D, M], f32, kind="ExternalInput")
    w = nc.dram_tensor("w", [D, NC_], f32, kind="ExternalInput")
    bias = nc.dram_tensor("bias", [P, NC_], f32, kind="ExternalInput")
    scales = nc.dram_tensor("scales", [P, 3], f32, kind="ExternalInput")
    out = nc.dram_tensor("out", [M, NC_], f32, kind="ExternalOutput")
    amax = nc.dram_tensor("amax", [P, 2], f32, kind="ExternalOutput")

    Copy = mybir.ActivationFunctionType.Copy
    AX = mybir.AxisListType
    Alu = mybir.AluOpType
    DR = mybir.MatmulPerfMode.DoubleRow

    MB = 256                 # x m-block width (2 m-tiles per load)
    NMB = M // MB            # 8 m-blocks
    NS = 512                 # W n-slice width
    XKQ = 8                  # k-chunks per x stage DMA  (4 DMAs per block)
    WKQ = 4                  # k-chunks per W stage DMA  (8 DMAs per slice)

    with tile.TileContext(nc) as tc:
        with (
            tc.tile_pool(name="const", bufs=1) as const,
            tc.tile_pool(name="xf8p", bufs=1) as xf8p,
            tc.tile_pool(name="wf8p", bufs=1) as wf8p,
            tc.tile_pool(name="stage", bufs=5) as stage,
            tc.tile_pool(name="opool", bufs=6) as opool,
            tc.tile_pool(name="psum", bufs=8, space="PSUM") as psum,
        ):
            bias_t = const.tile([P, NC_], f32)
            nc.sync.dma_start(bias_t[:], bias.ap())
            sc_t = const.tile([P, 3], f32)
            nc.sync.dma_start(sc_t[:], scales.ap())
            xa_t = const.tile([P, NMB * (KO // XKQ)], f32)
            wa_t = const.tile([P, NB * (KO // WKQ)], f32)
            amax_t = const.tile([P, 2], f32)

            # fp8 residents laid out so every conversion writes a contiguous
            # per-partition run (strided 3D destinations halve ACT rate)
            x_fp8 = xf8p.tile([P, NMB, KO, MB], fp8)
            w_fp8 = wf8p.tile([P, NB, KO, NS], fp8)

            def load_x_kq(mb, kq):
                xst = stage.tile([P, XKQ, MB], f32, tag="stg")
                nc.sync.dma_start(
                    xst[:],
                    xt.ap()[kq * XKQ * P:(kq + 1) * XKQ * P,
                            mb * MB:(mb + 1) * MB]
                        .rearrange("(ko p) m -> p ko m", p=P),
                )
                nc.scalar.activation(
                    x_fp8[:, mb, kq * XKQ:(kq + 1) * XKQ, :],
                    xst[:], Copy, scale=sc_t[:, 0:1])
                col = mb * (KO // XKQ) + kq
                nc.vector.tensor_reduce(xa_t[:, col:col + 1], xst[:],
                                        axis=AX.XY, op=Alu.max,
                                        apply_absolute_value=True)

            def load_x_block(mb):
                for kq in range(KO // XKQ):
                    load_x_kq(mb, kq)

            def load_w_kq(nb, kq):
                wst = stage.tile([P, WKQ, NS], f32, tag="stg")
                nc.sync.dma_start(
                    wst[:],
                    w.ap()[kq * WKQ * P:(kq + 1) * WKQ * P,
                           nb * NS:(nb + 1) * NS]
                        .rearrange("(ko p) n -> p ko n", p=P),
                )
                nc.scalar.activation(
                    w_fp8[:, nb, kq * WKQ:(kq + 1) * WKQ, :],
                    wst[:], Copy, scale=sc_t[:, 1:2])
                col = nb * (KO // WKQ) + kq
                nc.vector.tensor_reduce(wa_t[:, col:col + 1], wst[:],
                                        axis=AX.XY, op=Alu.max,
                                        apply_absolute_value=True)

            def load_w_slice(nb):
                for kq in range(KO // WKQ):
                    load_w_kq(nb, kq)

            def mm_group(mb, nb):
                # two m-tiles worth of full-K psum accumulation + eviction
                for mi in range(MB // P):
                    m0 = mb * MB + mi * P
                    ps = psum.tile([P, NS], f32)
                    for kp in range(KO // 2):
                        nc.tensor.matmul(
                            ps[:],
                            x_fp8[:, mb, 2 * kp:2 * kp + 2,
                                  mi * P:(mi + 1) * P],
                            w_fp8[:, nb, 2 * kp:2 * kp + 2, :],
                            start=(kp == 0),
                            stop=(kp == KO // 2 - 1),
                            perf_mode=DR,
                        )
                    ot = opool.tile([P, NS], f32)
                    nc.vector.tensor_scalar_mul(ot[:], ps[:], sc_t[:, 2:3])
                    nc.gpsimd.tensor_tensor(
                        ot[:], ot[:], bias_t[:, nb * NS:(nb + 1) * NS],
                        Alu.add)
                    nc.gpsimd.dma_start(
                        out.ap()[m0:m0 + P, nb * NS:(nb + 1) * NS], ot[:])

            # Balanced diagonal schedule: alternate W-slice / x-block loads
            # so matmul-group unlocks track input arrival roughly uniformly
            # (pure wave order back-loads PE work into the last wave, whose
            # inputs also land last).  After each load, emit every newly
            # unlocked (mb, nb) group.
            load_seq = [("w", 0), ("x", 0), ("x", 1),
                        ("w", 1), ("x", 2), ("x", 3),
                        ("w", 2), ("x", 4), ("x", 5),
                        ("w", 3), ("x", 6), ("x", 7)]
            loaded_w: list = []
            loaded_x: list = []
            for kind, idx in load_seq:
                if kind == "w":
                    load_w_slice(idx)
                    loaded_w.append(idx)
                    for mb in loaded_x:
                        mm_group(mb, idx)
                else:
                    load_x_block(idx)
                    loaded_x.append(idx)
                    for nb in loaded_w:
                        mm_group(idx, nb)

            # ---- final amax partials ----
            nc.vector.tensor_reduce(amax_t[:, 0:1], xa_t[:], axis=AX.X,
                                    op=Alu.max)
            nc.vector.tensor_reduce(amax_t[:, 1:2], wa_t[:], axis=AX.X,
                                    op=Alu.max)
            nc.sync.dma_start(amax.ap(), amax_t[:])

    nc.compile()
    return nc


def _get_nc():
    if "nc" not in _cached:
        _cached["nc"] = _build()
    return _cached["nc"]


def _qdq_flip_correction(out_full, x, w, s_in, s_k):
    """The device quantizes with x*(1/s) (f32 RNE) while the reference
    divides.  For a handful of elements that land exactly on (or within an
    ulp of) an e4m3 rounding midpoint the two differ by one fp8 ulp.  Find
    those elements on the host and apply the exact sparse correction
    Delta_out = s_in*s_k*(dqx @ qw_div + qx_mul @ dqw) in float64."""
    import ml_dtypes

    f8 = ml_dtypes.float8_e4m3

    def flips_and_q(t, s):
        rin = np.float32(1.0) / s
        q_mul = (t * rin).astype(f8)
        q_div = (t / s).astype(f8)
        fl = np.argwhere(q_mul.view(np.uint8) != q_div.view(np.uint8))
        return fl, q_mul, q_div

    fx, qx_mul, qx_div = flips_and_q(x, s_in)
    fw, qw_mul, qw_div = flips_and_q(w, s_k)
    if len(fx) == 0 and len(fw) == 0:
        return out_full

    ss = np.float64(s_in) * np.float64(s_k)
    corr = {}
    for m, k in fx:
        d = np.float64(qx_div[m, k]) - np.float64(qx_mul[m, k])
        row = corr.setdefault(("r", m), np.zeros(out_full.shape[1], np.float64))
        row += ss * d * qw_div[k, :].astype(np.float64)
    for k, n in fw:
        d = np.float64(qw_div[k, n]) - np.float64(qw_mul[k, n])
        col = corr.setdefault(("c", n), np.zeros(out_full.shape[0], np.float64))
        col += ss * d * qx_mul[:, k].astype(np.float64)
    for (kind, idx), v in corr.items():
        if kind == "r":
            out_full[idx, :] = (out_full[idx, :].astype(np.float64) + v).astype(np.float32)
        else:
            out_full[:, idx] = (out_full[:, idx].astype(np.float64) + v).astype(np.float32)
    return out_full


def kernel(x, kernel, bias, input_scale, kernel_scale, input_grad_scale,
           output_grad_scale, input_amax_history, kernel_amax_history,
           _trace=False):
    from concourse.bass_utils import run_bass_kernel_spmd

    x = np.asarray(x, dtype=np.float32)
    kernel = np.asarray(kernel, dtype=np.float32)
    bias = np.asarray(bias, dtype=np.float32)
    s_in = np.float32(np.asarray(input_scale))
    s_k = np.float32(np.asarray(kernel_scale))
    in_hist = np.asarray(input_amax_history, dtype=np.float32)
    k_hist = np.asarray(kernel_amax_history, dtype=np.float32)

    nc = _get_nc()

    scales_arr = np.empty((P, 3), dtype=np.float32)
    scales_arr[:, 0] = np.float32(1.0) / s_in
    scales_arr[:, 1] = np.float32(1.0) / s_k
    scales_arr[:, 2] = s_in * s_k

    xt_shards = [np.ascontiguousarray(x[r * M:(r + 1) * M, :].T)
                 for r in range(RG)]
    w_shards = [np.ascontiguousarray(kernel[:, c * NC_:(c + 1) * NC_])
                for c in range(CG)]
    bias_shards = [np.ascontiguousarray(
        np.broadcast_to(bias[c * NC_:(c + 1) * NC_], (P, NC_)))
        for c in range(CG)]

    in_maps = []
    for i in range(8):
        r, c = i // CG, i % CG
        in_maps.append({
            "xt": xt_shards[r],
            "w": w_shards[c],
            "bias": bias_shards[c],
            "scales": scales_arr,
        })

    res = run_bass_kernel_spmd(nc, in_maps, core_ids=list(range(8)),
                               trace=_trace)

    out_full = np.empty((N_ROWS, U), dtype=np.float32)
    amax_x = np.float32(0.0)
    amax_w = np.float32(0.0)
    for i in range(8):
        r, c = i // CG, i % CG
        out_full[r * M:(r + 1) * M, c * NC_:(c + 1) * NC_] = res.results[i]["out"]
        am = res.results[i]["amax"]
        amax_x = max(amax_x, am[:, 0].max())
        amax_w = max(amax_w, am[:, 1].max())

    out_full = _qdq_flip_correction(out_full, x, kernel, s_in, s_k)

    def updated_state(amax_cur, hist):
        new_hist = np.roll(hist, 1)
        new_hist[0] = amax_cur
        amax_eff = np.maximum(np.float32(new_hist.max()),
                              np.float32(2.0 ** (-10)))
        new_scale = np.float32(np.float32(1.1) * amax_eff) / np.float32(E4M3_MAX)
        return new_hist, np.float32(new_scale)

    new_in_hist, new_in_scale = updated_state(np.float32(amax_x), in_hist)
    new_k_hist, new_k_scale = updated_state(np.float32(amax_w), k_hist)

    if _trace:
        _cached["last_results"] = res  # stash for test harness introspection

    return out_full, new_in_scale, new_k_scale, new_in_hist, new_k_hist


# revision 48
# speedup vs baseline: 1.1506x; 1.1506x over previous
"""DenseFp8 forward kernel for 8 Trainium2 NeuronCores.

Reference semantics (per reference.py):
  qin  = qdq(x, s_in, 448, e4m3)        # round(x/s_in) on the e4m3 grid, * s_in
  qker = qdq(kernel, s_k, 448, e4m3)
  out  = qin @ qker + bias
  new_hist/new_scale bookkeeping from amax(|x|), amax(|kernel|)

Strategy:
  - 4 row-groups x 2 col-groups over the 8 cores. Core i handles x rows
    [r*2048,(r+1)*2048) and kernel cols [c*2048,(c+1)*2048), r=i//2, c=i%2.
  - Host pre-transposes each x row shard to xT [4096, 2048] so the
    contraction dim lands on SBUF partitions (TensorE matmul contracts
    over partitions for both operands).
  - On device: W f32 chunks stream in, are scaled by 1/s_k and cast to
    fp8e4 (TRN e4m3 == OCP e4m3fn on |v|<=240) into an SBUF-resident
    [128, 32, 2048] buffer.  x streams per 128-row m-tile, same
    conversion.  fp8 DoubleRow matmuls accumulate into PSUM.  Eviction:
    ScalarE copy * (s_in*s_k), VectorE adds the (host-broadcast) bias.
  - amax partials per partition are reduced on VectorE (abs_max) and
    shipped out as [128, 2]; final max + history roll happen on host.
  - Loads are emitted in a diagonal-wave order (W n-slice leads, then two
    x m-blocks per wave) with matmul groups emitted as soon as both of
    their operands are in flight, so the PE ramps up during the load
    phase instead of waiting for full residency.
  - Engine roles are split so the input path never queues behind
    MM-gated work: ScalarE does conversions only, VectorE does amax
    reduces + PSUM evictions, GpSimd does the bias add + output DMA.
    (Evictions or bias on ScalarE head-of-line block the conversions
    that feed the matmuls; measured +8-15us.)

Measured on 8 axon-tunneled trn2 cores: ~301-313 us HW exec (core 0;
occasional ~360 us outliers under shared-terminal contention), out rel
err ~6.8e-05 vs the fp32 jax reference, scale/history outputs
bit-exact.  PE matmul busy is ~225-233 us of that (~98% of the
157 TF/s fp8 DoubleRow roofline while streaming), with ~12 us of PE
gaps, ~41 us load prefix and ~11 us drain tail.
"""

import sys

if "/opt/trn_rl_repo" not in sys.path:
    sys.path.insert(0, "/opt/trn_rl_repo")

import numpy as np

P = 128
N_ROWS = 8192   # x rows
D = 4096        # contraction
U = 4096        # output cols
RG = 4          # row groups
CG = 2          # col groups
M = N_ROWS // RG      # 2048 rows per core
NC_ = U // CG         # 2048 cols per core
KO = D // P           # 32 k-chunks
MT = M // P           # 16 m-tiles
NB = NC_ // 512       # 4 psum banks per m-tile

AMAX_HIS_LEN = 16
E4M3_MAX = 448.0

_cached = {}


def _build():
    import concourse.mybir as mybir
    import concourse.tile as tile
    from concourse import bacc

    f32 = mybir.dt.float32
    fp8 = mybir.dt.float8e4

    nc = bacc.Bacc("TRN2", target_bir_lowering=False, debug=False, num_devices=8)

    xt = nc.dram_tensor("xt", [D, M], f32, kind="ExternalInput")
    w = nc.dram_tensor("w", [D, NC_], f32, kind="ExternalInput")
    bias = nc.dram_tensor("bias", [P, NC_], f32, kind="ExternalInput")
    scales = nc.dram_tensor("scales", [P, 3], f32, kind="ExternalInput")
    out = nc.dram_tensor("out", [M, NC_], f32, kind="ExternalOutput")
    amax = nc.dram_tensor("amax", [P, 2], f32, kind="ExternalOutput")

    Copy = mybir.ActivationFunctionType.Copy
    AX = mybir.AxisListType
    Alu = mybir.AluOpType
    DR = mybir.MatmulPerfMode.DoubleRow

    MB = 256                 # x m-block width (2 m-tiles per load)
    NMB = M // MB            # 8 m-blocks
    NS = 512                 # W n-slice width
    XKQ = 8                  # k-chunks per x stage DMA  (4 DMAs per block)
    WKQ = 4                  # k-chunks per W stage DMA  (8 DMAs per slice)

    with tile.TileContext(nc) as tc:
        with (
            tc.tile_pool(name="const", bufs=1) as const,
            tc.tile_pool(name="xf8p", bufs=1) as xf8p,
            tc.tile_pool(name="wf8p", bufs=1) as wf8p,
            tc.tile_pool(name="stage", bufs=5) as stage,
            tc.tile_pool(name="opool", bufs=6) as opool,
            tc.tile_pool(name="psum", bufs=8, space="PSUM") as psum,
        ):
            bias_t = const.tile([P, NC_], f32)
            nc.sync.dma_start(bias_t[:], bias.ap())
            sc_t = const.tile([P, 3], f32)
            nc.sync.dma_start(sc_t[:], scales.ap())
            xa_t = const.tile([P, NMB * (KO // XKQ)], f32)
            wa_t = const.tile([P, NB * (KO // WKQ)], f32)
            amax_t = const.tile([P, 2], f32)

            # fp8 residents laid out so every conversion writes a contiguous
            # per-partition run (strided 3D destinations halve ACT rate)
            x_fp8 = xf8p.tile([P, NMB, KO, MB], fp8)
            w_fp8 = wf8p.tile([P, NB, KO, NS], fp8)

            def load_x_kq(mb, kq):
                xst = stage.tile([P, XKQ, MB], f32, tag="stg")
                nc.sync.dma_start(
                    xst[:],
                    xt.ap()[kq * XKQ * P:(kq + 1) * XKQ * P,
                            mb * MB:(mb + 1) * MB]
                        .rearrange("(ko p) m -> p ko m", p=P),
                )
                nc.scalar.activation(
                    x_fp8[:, mb, kq * XKQ:(kq + 1) * XKQ, :],
                    xst[:], Copy, scale=sc_t[:, 0:1])
                col = mb * (KO // XKQ) + kq
                nc.vector.tensor_reduce(xa_t[:, col:col + 1], xst[:],
                                        axis=AX.XY, op=Alu.max,
                                        apply_absolute_value=True)

            def load_x_block(mb):
                for kq in range(KO // XKQ):
                    load_x_kq(mb, kq)

            def load_w_kq(nb, kq):
                wst = stage.tile([P, WKQ, NS], f32, tag="stg")
                nc.sync.dma_start(
                    wst[:],
                    w.ap()[kq * WKQ * P:(kq + 1) * WKQ * P,
                           nb * NS:(nb + 1) * NS]
                        .rearrange("(ko p) n -> p ko n", p=P),
                )
                nc.scalar.activation(
                    w_fp8[:, nb, kq * WKQ:(kq + 1) * WKQ, :],
                    wst[:], Copy, scale=sc_t[:, 1:2])
                col = nb * (KO // WKQ) + kq
                nc.vector.tensor_reduce(wa_t[:, col:col + 1], wst[:],
                                        axis=AX.XY, op=Alu.max,
                                        apply_absolute_value=True)

            def load_w_slice(nb):
                for kq in range(KO // WKQ):
                    load_w_kq(nb, kq)

            def mm_group(mb, nb):
                # two m-tiles worth of full-K psum accumulation + eviction
                for mi in range(MB // P):
                    m0 = mb * MB + mi * P
                    ps = psum.tile([P, NS], f32)
                    for kp in range(KO // 2):
                        nc.tensor.matmul(
                            ps[:],
                            x_fp8[:, mb, 2 * kp:2 * kp + 2,
                                  mi * P:(mi + 1) * P],
                            w_fp8[:, nb, 2 * kp:2 * kp + 2, :],
                            start=(kp == 0),
                            stop=(kp == KO // 2 - 1),
                            perf_mode=DR,
                        )
                    ot = opool.tile([P, NS], f32)
                    nc.vector.tensor_scalar_mul(ot[:], ps[:], sc_t[:, 2:3])
                    nc.gpsimd.tensor_tensor(
                        ot[:], ot[:], bias_t[:, nb * NS:(nb + 1) * NS],
                        Alu.add)
                    nc.gpsimd.dma_start(
                        out.ap()[m0:m0 + P, nb * NS:(nb + 1) * NS], ot[:])

            # Balanced diagonal schedule: alternate W-slice / x-block loads
            # so matmul-group unlocks track input arrival roughly uniformly
            # (pure wave order back-loads PE work into the last wave, whose
            # inputs also land last).  After each load, emit every newly
            # unlocked (mb, nb) group.
            load_seq = [("w", 0), ("x", 0), ("x", 1),
                        ("w", 1), ("x", 2), ("x", 3),
                        ("w", 2), ("x", 4), ("x", 5),
                        ("w", 3), ("x", 6), ("x", 7)]
            loaded_w: list = []
            loaded_x: list = []
            for kind, idx in load_seq:
                if kind == "w":
                    load_w_slice(idx)
                    loaded_w.append(idx)
                    for mb in loaded_x:
                        mm_group(mb, idx)
                else:
                    load_x_block(idx)
                    loaded_x.append(idx)
                    for nb in loaded_w:
                        mm_group(idx, nb)

            # ---- final amax partials ----
            nc.vector.tensor_reduce(amax_t[:, 0:1], xa_t[:], axis=AX.X,
                                    op=Alu.max)
            nc.vector.tensor_reduce(amax_t[:, 1:2], wa_t[:], axis=AX.X,
                                    op=Alu.max)
            nc.sync.dma_start(amax.ap(), amax_t[:])

    nc.compile()
    return nc


def _get_nc():
    if "nc" not in _cached:
        _cached["nc"] = _build()
    return _cached["nc"]


def _qdq_flip_correction(out_full, x, w, s_in, s_k):
    """The device quantizes with x*(1/s) (f32 RNE) while the reference
    divides.  For a handful of elements that land exactly on (or within an
    ulp of) an e4m3 rounding midpoint the two differ by one fp8 ulp.  Find
    those elements on the host and apply the exact sparse correction
    Delta_out = s_in*s_k*(dqx @ qw_div + qx_mul @ dqw) in float64."""
    import ml_dtypes

    f8 = ml_dtypes.float8_e4m3

    def flips_and_q(t, s):
        rin = np.float32(1.0) / s
        q_mul = (t * rin).astype(f8)
        q_div = (t / s).astype(f8)
        fl = np.argwhere(q_mul.view(np.uint8) != q_div.view(np.uint8))
        return fl, q_mul, q_div

    fx, qx_mul, qx_div = flips_and_q(x, s_in)
    fw, qw_mul, qw_div = flips_and_q(w, s_k)
    if len(fx) == 0 and len(fw) == 0:
        return out_full

    ss = np.float64(s_in) * np.float64(s_k)
    corr = {}
    for m, k in fx:
        d = np.float64(qx_div[m, k]) - np.float64(qx_mul[m, k])
        row = corr.setdefault(("r", m), np.zeros(out_full.shape[1], np.float64))
        row += ss * d * qw_div[k, :].astype(np.float64)
    for k, n in fw:
        d = np.float64(qw_div[k, n]) - np.float64(qw_mul[k, n])
        col = corr.setdefault(("c", n), np.zeros(out_full.shape[0], np.float64))
        col += ss * d * qx_mul[:, k].astype(np.float64)
    for (kind, idx), v in corr.items():
        if kind == "r":
            out_full[idx, :] = (out_full[idx, :].astype(np.float64) + v).astype(np.float32)
        else:
            out_full[:, idx] = (out_full[:, idx].astype(np.float64) + v).astype(np.float32)
    return out_full


def kernel(x, kernel, bias, input_scale, kernel_scale, input_grad_scale,
           output_grad_scale, input_amax_history, kernel_amax_history,
           _trace=False):
    from concourse.bass_utils import run_bass_kernel_spmd

    x = np.asarray(x, dtype=np.float32)
    kernel = np.asarray(kernel, dtype=np.float32)
    bias = np.asarray(bias, dtype=np.float32)
    s_in = np.float32(np.asarray(input_scale))
    s_k = np.float32(np.asarray(kernel_scale))
    in_hist = np.asarray(input_amax_history, dtype=np.float32)
    k_hist = np.asarray(kernel_amax_history, dtype=np.float32)

    nc = _get_nc()

    scales_arr = np.empty((P, 3), dtype=np.float32)
    scales_arr[:, 0] = np.float32(1.0) / s_in
    scales_arr[:, 1] = np.float32(1.0) / s_k
    scales_arr[:, 2] = s_in * s_k

    xt_shards = [np.ascontiguousarray(x[r * M:(r + 1) * M, :].T)
                 for r in range(RG)]
    w_shards = [np.ascontiguousarray(kernel[:, c * NC_:(c + 1) * NC_])
                for c in range(CG)]
    bias_shards = [np.ascontiguousarray(
        np.broadcast_to(bias[c * NC_:(c + 1) * NC_], (P, NC_)))
        for c in range(CG)]

    in_maps = []
    for i in range(8):
        r, c = i // CG, i % CG
        in_maps.append({
            "xt": xt_shards[r],
            "w": w_shards[c],
            "bias": bias_shards[c],
            "scales": scales_arr,
        })

    res = run_bass_kernel_spmd(nc, in_maps, core_ids=list(range(8)),
                               trace=_trace)

    out_full = np.empty((N_ROWS, U), dtype=np.float32)
    amax_x = np.float32(0.0)
    amax_w = np.float32(0.0)
    for i in range(8):
        r, c = i // CG, i % CG
        out_full[r * M:(r + 1) * M, c * NC_:(c + 1) * NC_] = res.results[i]["out"]
        am = res.results[i]["amax"]
        amax_x = max(amax_x, am[:, 0].max())
        amax_w = max(amax_w, am[:, 1].max())

    out_full = _qdq_flip_correction(out_full, x, kernel, s_in, s_k)

    def updated_state(amax_cur, hist):
        new_hist = np.roll(hist, 1)
        new_hist[0] = amax_cur
        amax_eff = np.maximum(np.float32(new_hist.max()),
                              np.float32(2.0 ** (-10)))
        new_scale = np.float32(np.float32(1.1) * amax_eff) / np.float32(E4M3_MAX)
        return new_hist, np.float32(new_scale)

    new_in_hist, new_in_scale = updated_state(np.float32(amax_x), in_hist)
    new_k_hist, new_k_scale = updated_state(np.float32(amax_w), k_hist)

    if _trace:
        _cached["last_results"] = res  # stash for test harness introspection

    return out_full, new_in_scale, new_k_scale, new_in_hist, new_k_hist


# revision 51
# speedup vs baseline: 1.1863x; 1.0310x over previous
"""DenseFp8 forward kernel for 8 Trainium2 NeuronCores.

Reference semantics (per reference.py):
  qin  = qdq(x, s_in, 448, e4m3)        # round(x/s_in) on the e4m3 grid, * s_in
  qker = qdq(kernel, s_k, 448, e4m3)
  out  = qin @ qker + bias
  new_hist/new_scale bookkeeping from amax(|x|), amax(|kernel|)

Strategy:
  - 4 row-groups x 2 col-groups over the 8 cores. Core i handles x rows
    [r*2048,(r+1)*2048) and kernel cols [c*2048,(c+1)*2048), r=i//2, c=i%2.
  - Host pre-transposes each x row shard to xT [4096, 2048] so the
    contraction dim lands on SBUF partitions (TensorE matmul contracts
    over partitions for both operands).
  - On device: W f32 chunks stream in, are scaled by 1/s_k and cast to
    fp8e4 (TRN e4m3 == OCP e4m3fn on |v|<=240) into an SBUF-resident
    [128, 32, 2048] buffer.  x streams per 128-row m-tile, same
    conversion.  fp8 DoubleRow matmuls accumulate into PSUM.  Eviction:
    ScalarE copy * (s_in*s_k), VectorE adds the (host-broadcast) bias.
  - amax partials per partition are reduced on VectorE (abs_max) and
    shipped out as [128, 2]; final max + history roll happen on host.
  - Loads are emitted in a diagonal-wave order (W n-slice leads, then two
    x m-blocks per wave) with matmul groups emitted as soon as both of
    their operands are in flight, so the PE ramps up during the load
    phase instead of waiting for full residency.
  - Engine roles are split so the input path never queues behind
    MM-gated work: ScalarE does conversions only, VectorE does amax
    reduces + PSUM evictions, GpSimd does the bias add + output DMA.
    (Evictions or bias on ScalarE head-of-line block the conversions
    that feed the matmuls; measured +8-15us.)

Measured on 8 axon-tunneled trn2 cores: ~301-313 us HW exec (core 0;
occasional ~360 us outliers under shared-terminal contention), out rel
err ~6.8e-05 vs the fp32 jax reference, scale/history outputs
bit-exact.  PE matmul busy is ~225-233 us of that (~98% of the
157 TF/s fp8 DoubleRow roofline while streaming), with ~12 us of PE
gaps, ~41 us load prefix and ~11 us drain tail.
"""

import sys

if "/opt/trn_rl_repo" not in sys.path:
    sys.path.insert(0, "/opt/trn_rl_repo")

import numpy as np

P = 128
N_ROWS = 8192   # x rows
D = 4096        # contraction
U = 4096        # output cols
RG = 4          # row groups
CG = 2          # col groups
M = N_ROWS // RG      # 2048 rows per core
NC_ = U // CG         # 2048 cols per core
KO = D // P           # 32 k-chunks
MT = M // P           # 16 m-tiles
NB = NC_ // 512       # 4 psum banks per m-tile

AMAX_HIS_LEN = 16
E4M3_MAX = 448.0

_cached = {}


def _build():
    import concourse.mybir as mybir
    import concourse.tile as tile
    from concourse import bacc

    f32 = mybir.dt.float32
    fp8 = mybir.dt.float8e4

    nc = bacc.Bacc("TRN2", target_bir_lowering=False, debug=False, num_devices=8)

    xt = nc.dram_tensor("xt", [D, M], f32, kind="ExternalInput")
    w = nc.dram_tensor("w", [D, NC_], f32, kind="ExternalInput")
    bias = nc.dram_tensor("bias", [P, NC_], f32, kind="ExternalInput")
    scales = nc.dram_tensor("scales", [P, 3], f32, kind="ExternalInput")
    out = nc.dram_tensor("out", [M, NC_], f32, kind="ExternalOutput")
    amax = nc.dram_tensor("amax", [P, 2], f32, kind="ExternalOutput")

    Copy = mybir.ActivationFunctionType.Copy
    AX = mybir.AxisListType
    Alu = mybir.AluOpType
    DR = mybir.MatmulPerfMode.DoubleRow

    MB = 256                 # x m-block width (2 m-tiles per load)
    NMB = M // MB            # 8 m-blocks
    NS = 512                 # W n-slice width
    XKQ = 8                  # k-chunks per x stage DMA  (4 DMAs per block)
    WKQ = 4                  # k-chunks per W stage DMA  (8 DMAs per slice)

    with tile.TileContext(nc) as tc:
        with (
            tc.tile_pool(name="const", bufs=1) as const,
            tc.tile_pool(name="xf8p", bufs=1) as xf8p,
            tc.tile_pool(name="wf8p", bufs=1) as wf8p,
            tc.tile_pool(name="stage", bufs=5) as stage,
            tc.tile_pool(name="stage0", bufs=2) as stage0,
            tc.tile_pool(name="opool", bufs=6) as opool,
            tc.tile_pool(name="psum", bufs=8, space="PSUM") as psum,
        ):
            bias_t = const.tile([P, NC_], f32)
            nc.sync.dma_start(bias_t[:], bias.ap())
            sc_t = const.tile([P, 3], f32)
            nc.sync.dma_start(sc_t[:], scales.ap())
            xa_t = const.tile([P, NMB * (KO // XKQ)], f32)
            wa_t = const.tile([P, NB * (KO // WKQ)], f32)
            amax_t = const.tile([P, 2], f32)

            # fp8 residents laid out so every conversion writes a contiguous
            # per-partition run (strided 3D destinations halve ACT rate)
            x_fp8 = xf8p.tile([P, NMB, KO, MB], fp8)
            w_fp8 = wf8p.tile([P, NB, KO, NS], fp8)

            def load_x_kq(mb, kq):
                xst = stage.tile([P, XKQ, MB], f32, tag="stg")
                nc.sync.dma_start(
                    xst[:],
                    xt.ap()[kq * XKQ * P:(kq + 1) * XKQ * P,
                            mb * MB:(mb + 1) * MB]
                        .rearrange("(ko p) m -> p ko m", p=P),
                )
                nc.scalar.activation(
                    x_fp8[:, mb, kq * XKQ:(kq + 1) * XKQ, :],
                    xst[:], Copy, scale=sc_t[:, 0:1])
                col = mb * (KO // XKQ) + kq
                nc.vector.tensor_reduce(xa_t[:, col:col + 1], xst[:],
                                        axis=AX.XY, op=Alu.max,
                                        apply_absolute_value=True)

            def load_x_block(mb):
                for kq in range(KO // XKQ):
                    load_x_kq(mb, kq)

            def load_x_block_fast(mb):
                # First x block: GpSimd DMA ring (idle early, runs parallel
                # to the W stream on the sync ring) + VectorE conversion
                # (idle early, while ScalarE chews the W slice), with a
                # dedicated 2-slot stage pool so it can't stall W's slots.
                for kq in range(KO // XKQ):
                    xst = stage0.tile([P, XKQ, MB], f32, tag="stg0")
                    nc.gpsimd.dma_start(
                        xst[:],
                        xt.ap()[kq * XKQ * P:(kq + 1) * XKQ * P,
                                mb * MB:(mb + 1) * MB]
                            .rearrange("(ko p) m -> p ko m", p=P),
                    )
                    nc.vector.tensor_scalar_mul(
                        x_fp8[:, mb, kq * XKQ:(kq + 1) * XKQ, :],
                        xst[:], sc_t[:, 0:1])
                    col = mb * (KO // XKQ) + kq
                    nc.vector.tensor_reduce(xa_t[:, col:col + 1], xst[:],
                                            axis=AX.XY, op=Alu.max,
                                            apply_absolute_value=True)

            def load_w_kq(nb, kq):
                wst = stage.tile([P, WKQ, NS], f32, tag="stg")
                nc.sync.dma_start(
                    wst[:],
                    w.ap()[kq * WKQ * P:(kq + 1) * WKQ * P,
                           nb * NS:(nb + 1) * NS]
                        .rearrange("(ko p) n -> p ko n", p=P),
                )
                nc.scalar.activation(
                    w_fp8[:, nb, kq * WKQ:(kq + 1) * WKQ, :],
                    wst[:], Copy, scale=sc_t[:, 1:2])
                col = nb * (KO // WKQ) + kq
                nc.vector.tensor_reduce(wa_t[:, col:col + 1], wst[:],
                                        axis=AX.XY, op=Alu.max,
                                        apply_absolute_value=True)

            def load_w_slice(nb):
                for kq in range(KO // WKQ):
                    load_w_kq(nb, kq)

            def mm_group(mb, nb):
                # two m-tiles worth of full-K psum accumulation + eviction
                for mi in range(MB // P):
                    m0 = mb * MB + mi * P
                    ps = psum.tile([P, NS], f32)
                    for kp in range(KO // 2):
                        nc.tensor.matmul(
                            ps[:],
                            x_fp8[:, mb, 2 * kp:2 * kp + 2,
                                  mi * P:(mi + 1) * P],
                            w_fp8[:, nb, 2 * kp:2 * kp + 2, :],
                            start=(kp == 0),
                            stop=(kp == KO // 2 - 1),
                            perf_mode=DR,
                        )
                    ot = opool.tile([P, NS], f32)
                    nc.vector.tensor_scalar_mul(ot[:], ps[:], sc_t[:, 2:3])
                    nc.gpsimd.tensor_tensor(
                        ot[:], ot[:], bias_t[:, nb * NS:(nb + 1) * NS],
                        Alu.add)
                    nc.gpsimd.dma_start(
                        out.ap()[m0:m0 + P, nb * NS:(nb + 1) * NS], ot[:])

            # Balanced diagonal schedule: alternate W-slice / x-block loads
            # so matmul-group unlocks track input arrival roughly uniformly
            # (pure wave order back-loads PE work into the last wave, whose
            # inputs also land last).  After each load, emit every newly
            # unlocked (mb, nb) group.
            load_seq = [("w", 0), ("x", 0), ("x", 1),
                        ("w", 1), ("x", 2), ("x", 3),
                        ("w", 2), ("x", 4), ("x", 5),
                        ("w", 3), ("x", 6), ("x", 7)]
            loaded_w: list = []
            loaded_x: list = []
            for kind, idx in load_seq:
                if kind == "w":
                    load_w_slice(idx)
                    loaded_w.append(idx)
                    for mb in loaded_x:
                        mm_group(mb, idx)
                else:
                    if idx == 0:
                        load_x_block_fast(idx)
                    else:
                        load_x_block(idx)
                    loaded_x.append(idx)
                    for nb in loaded_w:
                        mm_group(idx, nb)

            # ---- final amax partials ----
            nc.vector.tensor_reduce(amax_t[:, 0:1], xa_t[:], axis=AX.X,
                                    op=Alu.max)
            nc.vector.tensor_reduce(amax_t[:, 1:2], wa_t[:], axis=AX.X,
                                    op=Alu.max)
            nc.sync.dma_start(amax.ap(), amax_t[:])

    nc.compile()
    return nc


def _get_nc():
    if "nc" not in _cached:
        _cached["nc"] = _build()
    return _cached["nc"]


def _qdq_flip_correction(out_full, x, w, s_in, s_k):
    """The device quantizes with x*(1/s) (f32 RNE) while the reference
    divides.  For a handful of elements that land exactly on (or within an
    ulp of) an e4m3 rounding midpoint the two differ by one fp8 ulp.  Find
    those elements on the host and apply the exact sparse correction
    Delta_out = s_in*s_k*(dqx @ qw_div + qx_mul @ dqw) in float64."""
    import ml_dtypes

    f8 = ml_dtypes.float8_e4m3

    def flips_and_q(t, s):
        rin = np.float32(1.0) / s
        q_mul = (t * rin).astype(f8)
        q_div = (t / s).astype(f8)
        fl = np.argwhere(q_mul.view(np.uint8) != q_div.view(np.uint8))
        return fl, q_mul, q_div

    fx, qx_mul, qx_div = flips_and_q(x, s_in)
    fw, qw_mul, qw_div = flips_and_q(w, s_k)
    if len(fx) == 0 and len(fw) == 0:
        return out_full

    ss = np.float64(s_in) * np.float64(s_k)
    corr = {}
    for m, k in fx:
        d = np.float64(qx_div[m, k]) - np.float64(qx_mul[m, k])
        row = corr.setdefault(("r", m), np.zeros(out_full.shape[1], np.float64))
        row += ss * d * qw_div[k, :].astype(np.float64)
    for k, n in fw:
        d = np.float64(qw_div[k, n]) - np.float64(qw_mul[k, n])
        col = corr.setdefault(("c", n), np.zeros(out_full.shape[0], np.float64))
        col += ss * d * qx_mul[:, k].astype(np.float64)
    for (kind, idx), v in corr.items():
        if kind == "r":
            out_full[idx, :] = (out_full[idx, :].astype(np.float64) + v).astype(np.float32)
        else:
            out_full[:, idx] = (out_full[:, idx].astype(np.float64) + v).astype(np.float32)
    return out_full


def kernel(x, kernel, bias, input_scale, kernel_scale, input_grad_scale,
           output_grad_scale, input_amax_history, kernel_amax_history,
           _trace=False):
    from concourse.bass_utils import run_bass_kernel_spmd

    x = np.asarray(x, dtype=np.float32)
    kernel = np.asarray(kernel, dtype=np.float32)
    bias = np.asarray(bias, dtype=np.float32)
    s_in = np.float32(np.asarray(input_scale))
    s_k = np.float32(np.asarray(kernel_scale))
    in_hist = np.asarray(input_amax_history, dtype=np.float32)
    k_hist = np.asarray(kernel_amax_history, dtype=np.float32)

    nc = _get_nc()

    scales_arr = np.empty((P, 3), dtype=np.float32)
    scales_arr[:, 0] = np.float32(1.0) / s_in
    scales_arr[:, 1] = np.float32(1.0) / s_k
    scales_arr[:, 2] = s_in * s_k

    xt_shards = [np.ascontiguousarray(x[r * M:(r + 1) * M, :].T)
                 for r in range(RG)]
    w_shards = [np.ascontiguousarray(kernel[:, c * NC_:(c + 1) * NC_])
                for c in range(CG)]
    bias_shards = [np.ascontiguousarray(
        np.broadcast_to(bias[c * NC_:(c + 1) * NC_], (P, NC_)))
        for c in range(CG)]

    in_maps = []
    for i in range(8):
        r, c = i // CG, i % CG
        in_maps.append({
            "xt": xt_shards[r],
            "w": w_shards[c],
            "bias": bias_shards[c],
            "scales": scales_arr,
        })

    res = run_bass_kernel_spmd(nc, in_maps, core_ids=list(range(8)),
                               trace=_trace)

    out_full = np.empty((N_ROWS, U), dtype=np.float32)
    amax_x = np.float32(0.0)
    amax_w = np.float32(0.0)
    for i in range(8):
        r, c = i // CG, i % CG
        out_full[r * M:(r + 1) * M, c * NC_:(c + 1) * NC_] = res.results[i]["out"]
        am = res.results[i]["amax"]
        amax_x = max(amax_x, am[:, 0].max())
        amax_w = max(amax_w, am[:, 1].max())

    out_full = _qdq_flip_correction(out_full, x, kernel, s_in, s_k)

    def updated_state(amax_cur, hist):
        new_hist = np.roll(hist, 1)
        new_hist[0] = amax_cur
        amax_eff = np.maximum(np.float32(new_hist.max()),
                              np.float32(2.0 ** (-10)))
        new_scale = np.float32(np.float32(1.1) * amax_eff) / np.float32(E4M3_MAX)
        return new_hist, np.float32(new_scale)

    new_in_hist, new_in_scale = updated_state(np.float32(amax_x), in_hist)
    new_k_hist, new_k_scale = updated_state(np.float32(amax_w), k_hist)

    if _trace:
        _cached["last_results"] = res  # stash for test harness introspection

    return out_full, new_in_scale, new_k_scale, new_in_hist, new_k_hist
